# revision 11
# baseline (speedup 1.0000x reference)
"""Trainium2 Bass kernel for nn_Attention_27358941675773.

Reference computation (per batch b):
    q = x @ Q              [N, H]
    k = x @ K              [N, H]
    V = V_down @ V_up      [L, L]
    v = x @ V              [N, L]
    S = q @ k.T / 256      [N, N]
    out = softmax(S) @ v   [N, L]

Sharding: pure data-parallel over batch B=8 across the 8 NeuronCores
(one batch element per core); small params replicated. No collectives.

Per-core kernel design (N=4096, L=256, H=128):
  - Inputs shipped as fp16 (x transposed to [L, N]); all matmuls run at
    full PE rate. qT [H,N] / kT [H,N] are computed directly transposed so
    scores are built as S_T[m, n] (keys on partitions), no transposes.
  - Value path factored through the rank-H bottleneck:
        out = softmax(S) @ x @ V_down @ V_up
    so the O(N^2) product contracts into H=128 columns.
  - PSUM layout (8 banks): one 3-slot score ring [128, 3, 1024] f32
    (6 banks, manually indexed) + mid accumulator [128, 1024] f32
    (2 banks). exp runs on PAIRS of score tiles via a 3-dim AP over the
    ring (strides +4KB or -8KB), halving the per-instruction overhead on
    the Scalar engine, which paces the whole kernel.
  - Rowsum of exp-scores: a pairwise tree of 2048-wide bf16 adds on the
    Vector engine over the 15 leading pairs (17 ops/block instead of 31
    narrow ones), with the last pair folded separately so the
    post-last-exp chain is only two 1024-wide adds.
  - Partition-axis reduction+broadcast of the rowsum in ONE matmul with
    an all-ones [128,128] stationary operand (fp32r, full PE rate),
    replacing the 8.3us GpSimd PartitionAllReduce.
  - Normalization applied after V_up at the output-copy stage; the
    numerator copy (mid -> SBUF) rides the Vector engine, keeping the
    Scalar engine exclusively on the exp stream.
  - DMA: x arrives in 4 large descriptors on the SP queue; weights in 4
    single-issue strided descriptors on the GpSimd queue - the serialized
    ~0.7us-per-issue descriptor cost no longer delays the first exp.
  - Uniform half-block-lagged schedule as before: per pair-slot the PE
    runs 4 QK matmuls + 4 lagged attention@w matmuls; block 0 uses the
    projections as filler; ~10 junk matmuls warm the PE clock gate.
  - Output stored transposed [L, N] fp16; host un-transposes on gather.
"""

import os
import sys

import numpy as np

for _p in ("/opt/trn_rl_repo",):
    if _p not in sys.path and os.path.isdir(_p):
        sys.path.insert(0, _p)

B, N, L, H = 8, 4096, 256, 128
SCALER = 256.0
NB = 1024           # query-block (free dim of score tiles)
NBH = 512           # half tile (one PSUM bank of fp32)
NT = N // NB        # 4 query blocks
MT = N // 128       # 32 key tiles of 128
NP = 16             # key-tile PAIRS per block
P = 128


def _build():
    import concourse.bass as bass
    import concourse.tile as tile
    from concourse import bacc, bass_isa, mybir
    from contextlib import ExitStack

    f32 = mybir.dt.float32
    f32r = mybir.dt.float32r
    f16 = mybir.dt.float16
    bf16 = mybir.dt.bfloat16
    AF = mybir.ActivationFunctionType

    nc = bacc.Bacc(
        "TRN2", target_bir_lowering=False, debug=False, num_devices=B
    )

    xT_ext = nc.declare_dram_parameter("xT", [L, N], f16, isOutput=False)
    wq_ext = nc.declare_dram_parameter("Wq", [L, H], f16, isOutput=False)
    wk_ext = nc.declare_dram_parameter("Wk", [L, H], f16, isOutput=False)
    vd_ext = nc.declare_dram_parameter("Vd", [L, H], f16, isOutput=False)
    vu_ext = nc.declare_dram_parameter("Vu", [H, L], f16, isOutput=False)
    # output stored transposed [L, N]; host un-transposes at gather
    out_ext = nc.declare_dram_parameter("out", [L, N], f16, isOutput=True)

    with tile.TileContext(nc) as tc, ExitStack() as ctx:
        persist = ctx.enter_context(tc.tile_pool(name="persist", bufs=1))

        ones32f = persist.tile([P, P], f32)
        nc.gpsimd.memset(ones32f[:], 1.0)
        ones32 = persist.tile([P, P], f32r)
        nc.vector.tensor_copy(ones32[:], ones32f[:])
        # touch Exp right away so the ~2.7us ACT table load overlaps the
        # input DMAs instead of delaying the first real exp
        dum = persist.tile([1, 2], f32)
        nc.gpsimd.memset(dum[:], 0.0)
        nc.scalar.activation(dum[:, 1:2], dum[:, 0:1], AF.Exp)
        wrm = persist.tile([P, NBH], bf16, name="wrm")
        nc.vector.memset(wrm[:], 0.0)

        qw16 = persist.tile([P, 2 * H], f16)    # Q   [l_chunk][l_in, h]
        kw16 = persist.tile([P, 2 * H], f16)
        vd16 = persist.tile([P, 2 * H], f16)    # V_down [l_chunk][l_in, h]
        vu16 = persist.tile([P, L], f16)        # V_up   [h, l]
        vu_bf = persist.tile([P, L], bf16)      # V_up as bf16 (out matmul)
        xt16 = [persist.tile([P, N], f16, name=f"xt16_{c}") for c in range(2)]
        qT16 = persist.tile([P, N], f16)        # q.T       [h, n]
        kT16 = persist.tile([P, N], f16)        # k.T       [h, m]
        w_sb = persist.tile([P, MT * H], bf16)  # x@V_down  [m_tile][m_in, h]

        # ---------------- phase A: direct fp16 loads ----------------
        # x s0 chunks first (critical path for the first QK tiles), as
        # two large descriptors; the rest of x as two more. Weights ride
        # the GpSimd queue concurrently, one descriptor per tensor.
        for c in range(2):
            nc.sync.dma_start(
                xt16[c][:, 0:NB], xT_ext[c * P:(c + 1) * P, 0:NB]
            )
        for w_ext, w_sbuf in ((wq_ext, qw16), (wk_ext, kw16), (vd_ext, vd16)):
            nc.gpsimd.dma_start(
                w_sbuf[:].rearrange("p (c h) -> p c h", c=2),
                w_ext[:, :].rearrange("(c p) h -> p c h", c=2),
            )
        nc.gpsimd.dma_start(vu16[:], vu_ext[:, :])
        for c in range(2):
            nc.sync.dma_start(
                xt16[c][:, NB:N], xT_ext[c * P:(c + 1) * P, NB:N]
            )
        nc.vector.tensor_copy(vu_bf[:], vu16[:])

        # ------------- phases B+C: projections fused with attention -------
        with (
            tc.tile_pool(name="psum", bufs=1, space="PSUM") as psum_pool,
            tc.tile_pool(name="est", bufs=15) as est_pool,
            tc.tile_pool(name="tree", bufs=2) as tree_pool,
            tc.tile_pool(name="sb_small", bufs=2) as sb_small,
            tc.tile_pool(name="outfin", bufs=2) as outfin_pool,
        ):
            # manual PSUM layout: 3-slot score ring (6 banks) + aux (2
            # banks, time-shared between the mid accumulator and the
            # block-0 w/qkT projection staging)
            sr = psum_pool.tile([P, 3, NB], f32, name="score_ring")
            aux = psum_pool.tile([P, NB], f32, name="aux")
            gslot = [0]

            def slot():
                i = gslot[0] % 3
                gslot[0] += 1
                return i

            estP = {}     # (k, g) -> bf16 [128, 2048] pair exp tiles
            estS = {}     # (k, g) -> bf16 [128, 1024] single exp tiles
            mscs = {}     # k -> normalized mid (bf16, SBUF)
            bc = {}       # k -> [128, NB] f32 broadcast 1/rowsum (SBUF)
            tr = {}       # tree tiles by (k, name)

            def est_ap(k, t, h):
                # 512-wide slice of the exp tile holding key-tile t
                g, r = divmod(t, 3) if t < 30 else (10, t - 30)
                if g < 10 and r == 2:
                    return estS[(k, g)][:, h * NBH:(h + 1) * NBH]
                off = r * NB + h * NBH
                return estP[(k, g)][:, off:off + NBH]

            def proj_qkT_pair(w16, dst, f, where, on_act=False):
                # projects halves f and f+1; one copy. where: ring|aux
                ps = sr[:, slot(), :] if where == "ring" else aux[:, :]
                for half in range(2):
                    ff = f + half
                    for c in range(2):
                        nc.tensor.matmul(
                            ps[half * NBH:(half + 1) * NBH]
                            if False else ps[:, half * NBH:(half + 1) * NBH],
                            w16[:, c * H:(c + 1) * H],
                            xt16[c][:, ff * NBH:(ff + 1) * NBH],
                            start=(c == 0), stop=(c == 1),
                        )
                if on_act:
                    nc.scalar.activation(
                        dst[:, f * NBH:(f + 2) * NBH], ps, AF.Copy
                    )
                else:
                    nc.vector.tensor_copy(dst[:, f * NBH:(f + 2) * NBH], ps)

            def proj_w_batch(b):
                # w tiles 4b..4b+3 staged in an aux half, one copy
                ps = aux[:, (b % 2) * NBH:(b % 2 + 1) * NBH]
                for j4 in range(4):
                    j = 4 * b + j4
                    for c in range(2):
                        nc.tensor.matmul(
                            ps[:, j4 * H:(j4 + 1) * H],
                            xt16[c][:, j * P:(j + 1) * P],
                            vd16[:, c * H:(c + 1) * H],
                            start=(c == 0), stop=(c == 1),
                        )
                nc.vector.tensor_copy(
                    w_sb[:, b * NBH:(b + 1) * NBH], ps
                )

            def qk_exp_P(k, g):
                t0 = 3 * g if g < 10 else 30
                i0 = slot()
                i1 = slot()
                for t, i in ((t0, i0), (t0 + 1, i1)):
                    for h in range(2):
                        nc.tensor.matmul(
                            sr[:, i, h * NBH:(h + 1) * NBH],
                            kT16[:, t * P:(t + 1) * P],
                            qT16[:, k * NB + h * NBH: k * NB + (h + 1) * NBH],
                            start=True, stop=True,
                        )
                if i1 == i0 + 1:
                    src_ap = sr[:, i0:i0 + 2, :]
                else:          # (2, 0): stride -2 pair
                    src_ap = sr[:, i0::-2, :]
                e = est_pool.tile([P, 2 * NB], bf16, tag="estP", bufs=15,
                                  name=f"estP_{k}_{g}")
                estP[(k, g)] = e
                nc.scalar.activation(e[:], src_ap, AF.Exp, scale=1.0 / SCALER)

            def qk_exp_S(k, g):
                t = 3 * g + 2
                i = slot()
                for h in range(2):
                    nc.tensor.matmul(
                        sr[:, i, h * NBH:(h + 1) * NBH],
                        kT16[:, t * P:(t + 1) * P],
                        qT16[:, k * NB + h * NBH: k * NB + (h + 1) * NBH],
                        start=True, stop=True,
                    )
                e = est_pool.tile([P, NB], bf16, tag="estS", bufs=13,
                                  name=f"estS_{k}_{g}")
                estS[(k, g)] = e
                nc.scalar.activation(e[:], sr[:, i, :], AF.Exp,
                                     scale=1.0 / SCALER)

            # ---- PV: FIFO queue of (k, j), popped on a per-group budget
            pvq = []
            pvhead = [0]

            def norm_mid(k):
                msc = sb_small.tile([P, NB], bf16, tag="msc", bufs=2,
                                    name=f"msc_{k}")
                nc.vector.tensor_copy(msc[:], aux[:, :])
                mscs[k] = msc

            def emit_pv(n):
                for _ in range(n):
                    if pvhead[0] >= len(pvq):
                        return
                    kk, j = pvq[pvhead[0]]
                    pvhead[0] += 1
                    for h in range(2):
                        nc.tensor.matmul(
                            aux[:, h * NBH:(h + 1) * NBH],
                            w_sb[:, j * H:(j + 1) * H],
                            est_ap(kk, j, h),
                            start=(j == 0), stop=(j == MT - 1),
                        )
                    if j == MT - 1:
                        norm_mid(kk)

            def tadd(k, name, a, b, dtype, width):
                nb = 1 if name in ("c0", "pp", "sss", "sp", "part") else 2
                t = tree_pool.tile([P, width], dtype, tag=name.rstrip(
                    "0123456789") or name, bufs=nb, name=f"{name}_{k}")
                nc.vector.tensor_add(t[:], a, b)
                tr[(k, name)] = t
                return t

            def tree_adds(k, g):
                # P-chain (2048-wide) + S-chain (1024-wide), bf16
                W2, W1 = 2 * NB, NB
                if g % 2 == 1:
                    i = g // 2
                    tadd(k, f"a{i}", estP[(k, g - 1)][:], estP[(k, g)][:],
                         bf16, W2)
                    tadd(k, f"s{i}", estS[(k, g - 1)][:], estS[(k, g)][:],
                         bf16, W1)
                if g == 3:
                    tadd(k, "b0", tr[(k, "a0")][:], tr[(k, "a1")][:], bf16, W2)
                    tadd(k, "ss0", tr[(k, "s0")][:], tr[(k, "s1")][:], bf16, W1)
                if g == 7:
                    tadd(k, "b1", tr[(k, "a2")][:], tr[(k, "a3")][:], bf16, W2)
                    tadd(k, "ss1", tr[(k, "s2")][:], tr[(k, "s3")][:], bf16, W1)
                    tadd(k, "c0", tr[(k, "b0")][:], tr[(k, "b1")][:], bf16, W2)
                if g == 9:
                    pp = tadd(k, "pp", tr[(k, "c0")][:], tr[(k, "a4")][:],
                              bf16, W2)
                    tadd(k, "sss", tr[(k, "ss0")][:], tr[(k, "ss1")][:],
                         bf16, W1)
                    sp = tadd(k, "sp", tr[(k, "sss")][:], tr[(k, "s4")][:],
                              bf16, W1)
                    pf = tree_pool.tile([P, NB], f32, tag="pf", bufs=1,
                                        name=f"pf_{k}")
                    nc.vector.tensor_add(pf[:], pp[:, 0:NB], pp[:, NB:2 * NB])
                    tr[(k, "pf")] = pf
                if g == 10:
                    tadd(k, "part", tr[(k, "pf")][:], tr[(k, "sp")][:],
                         f32, W1)

            def fold_last(k):
                # fold the final pair (tiles 30,31) into the rowsum
                p10f = tree_pool.tile([P, NB], f32, tag="p10f", bufs=1,
                                      name=f"p10f_{k}")
                nc.vector.tensor_add(
                    p10f[:], estP[(k, 10)][:, 0:NB], estP[(k, 10)][:, NB:2 * NB]
                )
                t = tree_pool.tile([P, NB], f32r, tag="t5", bufs=1,
                                   name=f"t5_{k}")
                nc.vector.tensor_add(t[:], tr[(k, "part")][:], p10f[:])
                tr[(k, "t5")] = t

            def bc_chain(k):
                # partition-sum + broadcast in one all-ones fp32r matmul
                i = slot()
                for h in range(2):
                    nc.tensor.matmul(
                        sr[:, i, h * NBH:(h + 1) * NBH],
                        ones32[:],
                        tr[(k, "t5")][:, h * NBH:(h + 1) * NBH],
                        start=True, stop=True,
                    )
                bck = sb_small.tile([P, NB], f32, tag="bc", bufs=2,
                                    name=f"bc_{k}")
                nc.vector.reciprocal_approx_fast(bck[:], sr[:, i, :])
                bc[k] = bck

            def drain_out(k):
                for lt in range(2):
                    i = slot()
                    for h in range(2):
                        nc.tensor.matmul(
                            sr[:, i, h * NBH:(h + 1) * NBH],
                            vu_bf[:, lt * P:(lt + 1) * P],
                            mscs[k][:, h * NBH:(h + 1) * NBH],
                            start=True, stop=True,
                        )
                    fin = outfin_pool.tile([P, NB], f16, tag="fin")
                    nc.vector.tensor_mul(fin[:], sr[:, i, :], bc[k][:])
                    nc.gpsimd.dma_start(
                        out_ext[lt * P:(lt + 1) * P, k * NB:(k + 1) * NB],
                        fin[:],
                    )

            # PE warm-up while the x DMA is in flight
            for _ in range(10):
                i = slot()
                nc.tensor.matmul(
                    sr[:, i, 0:NBH], wrm[:, :P], wrm[:], start=True, stop=True
                )

            # head: first QK tiles need qT/kT half-blocks 0,1 (chunk s0)
            proj_qkT_pair(qw16, qT16, 0, "ring", on_act=True)
            proj_qkT_pair(kw16, kT16, 0, "ring", on_act=False)

            # per-group PV budgets: 32 js per block, half-block lag
            BUD = [3, 3, 3, 3, 4, 3, 3, 3, 3, 4, 0]
            BUD0 = [0, 0, 0, 0, 0, 3, 3, 3, 3, 4, 0]
            BUD3 = [3, 3, 3, 3, 4, 3, 5, 6, 6, 7, 2]

            for k in range(NT):
                pvq.extend((k, j) for j in range(MT))
                bud = BUD0 if k == 0 else (BUD3 if k == NT - 1 else BUD)
                for g in range(11):
                    emit_pv(bud[g])
                    qk_exp_P(k, g)
                    if g < 10:
                        qk_exp_S(k, g)
                    if k == 0:
                        # projection fillers: w batches + late qkT halves
                        if g <= 3:
                            proj_w_batch(2 * g)
                            proj_w_batch(2 * g + 1)
                        if g == 1:
                            proj_qkT_pair(kw16, kT16, 2, "aux")
                        if g == 3:
                            proj_qkT_pair(kw16, kT16, 4, "aux")
                        if g == 5:
                            proj_qkT_pair(kw16, kT16, 6, "ring")
                        if g == 6:
                            proj_qkT_pair(qw16, qT16, 2, "ring")
                        if g == 8:
                            proj_qkT_pair(qw16, qT16, 4, "ring")
                    if k == 1 and g == 1:
                        proj_qkT_pair(qw16, qT16, 6, "ring")
                    if k >= 1:
                        if g == 0:
                            fold_last(k - 1)
                        if g == 1:
                            bc_chain(k - 1)
                        if g == 5:
                            drain_out(k - 1)
                    tree_adds(k, g)

            # epilogue: drain the PV queue, block-3 rowsum chain, output
            k3 = NT - 1
            emit_pv(len(pvq) - pvhead[0])
            fold_last(k3)
            bc_chain(k3)
            drain_out(k3)

    if not nc.is_finalized():
        nc.finalize()
    return nc


_GRAPH_CACHE = {}


def _get_graph():
    if "nc" not in _GRAPH_CACHE:
        _GRAPH_CACHE["nc"] = _build()
    return _GRAPH_CACHE["nc"]


def run(inputs: dict, trace: bool = False):
    """Run the SPMD kernel on 8 cores. Returns (output, BassKernelResults)."""
    from concourse.bass_utils import run_bass_kernel_spmd

    x = np.asarray(inputs["x"], dtype=np.float32)
    Q = np.asarray(inputs["Q"], dtype=np.float32)[0]
    K = np.asarray(inputs["K"], dtype=np.float32)[0]
    Vd = np.asarray(inputs["V_down"], dtype=np.float32)[0]
    Vu = np.asarray(inputs["V_up"], dtype=np.float32)[0]

    wq = np.ascontiguousarray(Q).astype(np.float16)
    wk = np.ascontiguousarray(K).astype(np.float16)
    vd = np.ascontiguousarray(Vd).astype(np.float16)
    vu = np.ascontiguousarray(Vu).astype(np.float16)

    in_maps = []
    for b in range(B):
        in_maps.append({
            "xT": np.ascontiguousarray(x[b].T).astype(np.float16),
            "Wq": wq,
            "Wk": wk,
            "Vd": vd,
            "Vu": vu,
        })

    nc = _get_graph()
    res = run_bass_kernel_spmd(nc, in_maps, core_ids=list(range(B)), trace=trace)
    # device output is [L, N] per core; un-transpose during the gather
    out = np.stack([np.asarray(res.results[i]["out"]).astype(np.float32).T for i in range(B)])
    return np.ascontiguousarray(out, dtype=np.float32), res


def kernel(**inputs) -> np.ndarray:
    out, _ = run(inputs, trace=False)
    return out


# revision 12
# speedup vs baseline: 1.1882x; 1.1882x over previous
"""Trainium2 Bass kernel for nn_Attention_27358941675773.

Reference computation (per batch b):
    q = x @ Q              [N, H]
    k = x @ K              [N, H]
    V = V_down @ V_up      [L, L]
    v = x @ V              [N, L]
    S = q @ k.T / 256      [N, N]
    out = softmax(S) @ v   [N, L]

Sharding: pure data-parallel over batch B=8 across the 8 NeuronCores
(one batch element per core); small params replicated. No collectives.

Per-core kernel design (N=4096, L=256, H=128):
  - Inputs shipped as fp16 (x transposed to [L, N]); all matmuls run at
    full PE rate. qT [H,N] / kT [H,N] are computed directly transposed so
    scores are built as S_T[m, n] (keys on partitions), no transposes.
  - Value path factored through the rank-H bottleneck:
        out = softmax(S) @ x @ V_down @ V_up
    so the O(N^2) product contracts into H=128 columns.
  - PSUM layout (8 banks): one 3-slot score ring [128, 3, 1024] f32
    (6 banks, manually indexed) + mid accumulator [128, 1024] f32
    (2 banks). exp runs on PAIRS of score tiles via a 3-dim AP over the
    ring (strides +4KB or -8KB), halving the per-instruction overhead on
    the Scalar engine, which paces the whole kernel.
  - Rowsum of exp-scores: a pairwise tree of 2048-wide bf16 adds on the
    Vector engine over the 15 leading pairs (17 ops/block instead of 31
    narrow ones), with the last pair folded separately so the
    post-last-exp chain is only two 1024-wide adds.
  - Partition-axis reduction+broadcast of the rowsum in ONE matmul with
    an all-ones [128,128] stationary operand (fp32r, full PE rate),
    replacing the 8.3us GpSimd PartitionAllReduce.
  - Normalization applied after V_up at the output-copy stage; the
    numerator copy (mid -> SBUF) rides the Vector engine, keeping the
    Scalar engine exclusively on the exp stream.
  - DMA: x arrives in 4 large descriptors on the SP queue; weights in 4
    single-issue strided descriptors on the GpSimd queue - the serialized
    ~0.7us-per-issue descriptor cost no longer delays the first exp.
  - Uniform half-block-lagged schedule as before: per pair-slot the PE
    runs 4 QK matmuls + 4 lagged attention@w matmuls; block 0 uses the
    projections as filler; ~10 junk matmuls warm the PE clock gate.
  - Output stored transposed [L, N] fp16; host un-transposes on gather.
"""

import os
import sys

import numpy as np

for _p in ("/opt/trn_rl_repo",):
    if _p not in sys.path and os.path.isdir(_p):
        sys.path.insert(0, _p)

B, N, L, H = 8, 4096, 256, 128
SCALER = 256.0
NB = 1024           # query-block (free dim of score tiles)
NBH = 512           # half tile (one PSUM bank of fp32)
NT = N // NB        # 4 query blocks
MT = N // 128       # 32 key tiles of 128
NP = 16             # key-tile PAIRS per block
P = 128


def _build():
    import concourse.bass as bass
    import concourse.tile as tile
    from concourse import bacc, bass_isa, mybir
    from contextlib import ExitStack

    f32 = mybir.dt.float32
    f32r = mybir.dt.float32r
    f16 = mybir.dt.float16
    bf16 = mybir.dt.bfloat16
    AF = mybir.ActivationFunctionType

    nc = bacc.Bacc(
        "TRN2", target_bir_lowering=False, debug=False, num_devices=B
    )

    xT_ext = nc.declare_dram_parameter("xT", [L, N], f16, isOutput=False)
    wq_ext = nc.declare_dram_parameter("Wq", [L, H], f16, isOutput=False)
    wk_ext = nc.declare_dram_parameter("Wk", [L, H], f16, isOutput=False)
    vd_ext = nc.declare_dram_parameter("Vd", [L, H], f16, isOutput=False)
    vu_ext = nc.declare_dram_parameter("Vu", [H, L], f16, isOutput=False)
    # output stored transposed [L, N]; host un-transposes at gather
    out_ext = nc.declare_dram_parameter("out", [L, N], f16, isOutput=True)

    with tile.TileContext(nc) as tc, ExitStack() as ctx:
        persist = ctx.enter_context(tc.tile_pool(name="persist", bufs=1))

        ones32f = persist.tile([P, P], f32)
        nc.gpsimd.memset(ones32f[:], 1.0)
        ones32 = persist.tile([P, P], f32r)
        nc.vector.tensor_copy(ones32[:], ones32f[:])
        # touch Exp right away so the ~2.7us ACT table load overlaps the
        # input DMAs instead of delaying the first real exp
        dum = persist.tile([1, 2], f32)
        nc.gpsimd.memset(dum[:], 0.0)
        nc.scalar.activation(dum[:, 1:2], dum[:, 0:1], AF.Exp)
        wrm = persist.tile([P, NBH], bf16, name="wrm")
        nc.vector.memset(wrm[:], 0.0)

        qw16 = persist.tile([P, 2 * H], f16)    # Q   [l_chunk][l_in, h]
        kw16 = persist.tile([P, 2 * H], f16)
        vd16 = persist.tile([P, 2 * H], f16)    # V_down [l_chunk][l_in, h]
        vu16 = persist.tile([P, L], f16)        # V_up   [h, l]
        vu_bf = persist.tile([P, L], bf16)      # V_up as bf16 (out matmul)
        xt16 = [persist.tile([P, N], f16, name=f"xt16_{c}") for c in range(2)]
        qT16 = persist.tile([P, N], f16)        # q.T       [h, n]
        kT16 = persist.tile([P, N], f16)        # k.T       [h, m]
        w_sb = persist.tile([P, MT * H], bf16)  # x@V_down  [m_tile][m_in, h]

        # ---------------- phase A: direct fp16 loads ----------------
        # x s0 chunks first (critical path for the first QK tiles), as
        # two large descriptors; the rest of x as two more. Weights ride
        # the GpSimd queue concurrently, one descriptor per tensor.
        for c in range(2):
            nc.sync.dma_start(
                xt16[c][:, 0:NB], xT_ext[c * P:(c + 1) * P, 0:NB]
            )
        for w_ext, w_sbuf in ((wq_ext, qw16), (wk_ext, kw16), (vd_ext, vd16)):
            nc.gpsimd.dma_start(
                w_sbuf[:].rearrange("p (c h) -> p c h", c=2),
                w_ext[:, :].rearrange("(c p) h -> p c h", c=2),
            )
        nc.gpsimd.dma_start(vu16[:], vu_ext[:, :])
        for c in range(2):
            nc.sync.dma_start(
                xt16[c][:, NB:N], xT_ext[c * P:(c + 1) * P, NB:N]
            )
        nc.vector.tensor_copy(vu_bf[:], vu16[:])

        # ------------- phases B+C: projections fused with attention -------
        with (
            tc.tile_pool(name="psum", bufs=1, space="PSUM") as psum_pool,
            tc.tile_pool(name="est", bufs=15) as est_pool,
            tc.tile_pool(name="tree", bufs=2) as tree_pool,
            tc.tile_pool(name="sb_small", bufs=2) as sb_small,
            tc.tile_pool(name="outfin", bufs=2) as outfin_pool,
        ):
            # manual PSUM layout: 3-slot score ring (6 banks) + aux (2
            # banks, time-shared between the mid accumulator and the
            # block-0 w/qkT projection staging)
            sr = psum_pool.tile([P, 3, NB], f32, name="score_ring")
            aux = psum_pool.tile([P, NB], f32, name="aux")
            gslot = [0]

            def slot():
                i = gslot[0] % 3
                gslot[0] += 1
                return i

            estP = {}     # (k, g) -> bf16 [128, 2048] pair exp tiles
            estS = {}     # (k, g) -> bf16 [128, 1024] single exp tiles
            mscs = {}     # k -> normalized mid (bf16, SBUF)
            bc = {}       # k -> [128, NB] f32 broadcast 1/rowsum (SBUF)
            tr = {}       # tree tiles by (k, name)

            def est_ap(k, t, h):
                # 512-wide slice of the exp tile holding key-tile t
                g, r = divmod(t, 3) if t < 30 else (10, t - 30)
                if g < 10 and r == 2:
                    return estS[(k, g)][:, h * NBH:(h + 1) * NBH]
                off = r * NB + h * NBH
                return estP[(k, g)][:, off:off + NBH]

            def proj_qkT_pair(w16, dst, f, where, on_act=False):
                # projects halves f and f+1; one copy. where: ring|aux
                ps = sr[:, slot(), :] if where == "ring" else aux[:, :]
                for half in range(2):
                    ff = f + half
                    for c in range(2):
                        nc.tensor.matmul(
                            ps[half * NBH:(half + 1) * NBH]
                            if False else ps[:, half * NBH:(half + 1) * NBH],
                            w16[:, c * H:(c + 1) * H],
                            xt16[c][:, ff * NBH:(ff + 1) * NBH],
                            start=(c == 0), stop=(c == 1),
                        )
                if on_act:
                    nc.scalar.activation(
                        dst[:, f * NBH:(f + 2) * NBH], ps, AF.Copy
                    )
                else:
                    nc.vector.tensor_copy(dst[:, f * NBH:(f + 2) * NBH], ps)

            def proj_w_batch(b):
                # w tiles 4b..4b+3 staged in an aux half, one copy
                ps = aux[:, (b % 2) * NBH:(b % 2 + 1) * NBH]
                for j4 in range(4):
                    j = 4 * b + j4
                    for c in range(2):
                        nc.tensor.matmul(
                            ps[:, j4 * H:(j4 + 1) * H],
                            xt16[c][:, j * P:(j + 1) * P],
                            vd16[:, c * H:(c + 1) * H],
                            start=(c == 0), stop=(c == 1),
                        )
                nc.vector.tensor_copy(
                    w_sb[:, b * NBH:(b + 1) * NBH], ps
                )

            def qk_exp_P(k, g):
                t0 = 3 * g if g < 10 else 30
                # keep P-pairs contiguous -- a (2,0) wrap pair has a
                # bounding AP range spanning all three slots, which the
                # overlap tracker treats as a full-ring dependency
                if gslot[0] % 3 == 2:
                    gslot[0] += 1
                i0 = slot()
                i1 = slot()
                for t, i in ((t0, i0), (t0 + 1, i1)):
                    for h in range(2):
                        nc.tensor.matmul(
                            sr[:, i, h * NBH:(h + 1) * NBH],
                            kT16[:, t * P:(t + 1) * P],
                            qT16[:, k * NB + h * NBH: k * NB + (h + 1) * NBH],
                            start=True, stop=True,
                        )
                src_ap = sr[:, i0:i0 + 2, :]
                e = est_pool.tile([P, 2 * NB], bf16, tag="estP", bufs=15,
                                  name=f"estP_{k}_{g}")
                estP[(k, g)] = e
                nc.scalar.activation(e[:], src_ap, AF.Exp, scale=1.0 / SCALER)

            def qk_exp_S(k, g):
                t = 3 * g + 2
                i = slot()
                for h in range(2):
                    nc.tensor.matmul(
                        sr[:, i, h * NBH:(h + 1) * NBH],
                        kT16[:, t * P:(t + 1) * P],
                        qT16[:, k * NB + h * NBH: k * NB + (h + 1) * NBH],
                        start=True, stop=True,
                    )
                e = est_pool.tile([P, NB], bf16, tag="estS", bufs=13,
                                  name=f"estS_{k}_{g}")
                estS[(k, g)] = e
                nc.scalar.activation(e[:], sr[:, i, :], AF.Exp,
                                     scale=1.0 / SCALER)

            # ---- PV: FIFO queue of (k, j), popped on a per-group budget
            pvq = []
            pvhead = [0]

            def norm_mid(k):
                msc = sb_small.tile([P, NB], bf16, tag="msc", bufs=2,
                                    name=f"msc_{k}")
                nc.vector.tensor_copy(msc[:], aux[:, :])
                mscs[k] = msc

            def emit_pv(n):
                for _ in range(n):
                    if pvhead[0] >= len(pvq):
                        return
                    kk, j = pvq[pvhead[0]]
                    pvhead[0] += 1
                    for h in range(2):
                        nc.tensor.matmul(
                            aux[:, h * NBH:(h + 1) * NBH],
                            w_sb[:, j * H:(j + 1) * H],
                            est_ap(kk, j, h),
                            start=(j == 0), stop=(j == MT - 1),
                        )
                    if j == MT - 1:
                        norm_mid(kk)

            def tadd(k, name, a, b, dtype, width):
                nb = 1 if name in ("c0", "pp", "sss", "sp", "part") else 2
                t = tree_pool.tile([P, width], dtype, tag=name.rstrip(
                    "0123456789") or name, bufs=nb, name=f"{name}_{k}")
                nc.vector.tensor_add(t[:], a, b)
                tr[(k, name)] = t
                return t

            def tree_adds(k, g):
                # P-chain (2048-wide) + S-chain (1024-wide), bf16
                W2, W1 = 2 * NB, NB
                if g % 2 == 1:
                    i = g // 2
                    tadd(k, f"a{i}", estP[(k, g - 1)][:], estP[(k, g)][:],
                         bf16, W2)
                    tadd(k, f"s{i}", estS[(k, g - 1)][:], estS[(k, g)][:],
                         bf16, W1)
                if g == 3:
                    tadd(k, "b0", tr[(k, "a0")][:], tr[(k, "a1")][:], bf16, W2)
                    tadd(k, "ss0", tr[(k, "s0")][:], tr[(k, "s1")][:], bf16, W1)
                if g == 7:
                    tadd(k, "b1", tr[(k, "a2")][:], tr[(k, "a3")][:], bf16, W2)
                    tadd(k, "ss1", tr[(k, "s2")][:], tr[(k, "s3")][:], bf16, W1)
                    tadd(k, "c0", tr[(k, "b0")][:], tr[(k, "b1")][:], bf16, W2)
                if g == 9:
                    pp = tadd(k, "pp", tr[(k, "c0")][:], tr[(k, "a4")][:],
                              bf16, W2)
                    tadd(k, "sss", tr[(k, "ss0")][:], tr[(k, "ss1")][:],
                         bf16, W1)
                    sp = tadd(k, "sp", tr[(k, "sss")][:], tr[(k, "s4")][:],
                              bf16, W1)
                    pf = tree_pool.tile([P, NB], f32, tag="pf", bufs=1,
                                        name=f"pf_{k}")
                    nc.vector.tensor_add(pf[:], pp[:, 0:NB], pp[:, NB:2 * NB])
                    tr[(k, "pf")] = pf
                if g == 10:
                    tadd(k, "part", tr[(k, "pf")][:], tr[(k, "sp")][:],
                         f32, W1)

            def fold_last(k):
                # fold the final pair (tiles 30,31) into the rowsum
                p10f = tree_pool.tile([P, NB], f32, tag="p10f", bufs=1,
                                      name=f"p10f_{k}")
                nc.vector.tensor_add(
                    p10f[:], estP[(k, 10)][:, 0:NB], estP[(k, 10)][:, NB:2 * NB]
                )
                t = tree_pool.tile([P, NB], f32r, tag="t5", bufs=1,
                                   name=f"t5_{k}")
                nc.vector.tensor_add(t[:], tr[(k, "part")][:], p10f[:])
                tr[(k, "t5")] = t

            def bc_chain(k):
                # partition-sum + broadcast in one all-ones fp32r matmul
                i = slot()
                for h in range(2):
                    nc.tensor.matmul(
                        sr[:, i, h * NBH:(h + 1) * NBH],
                        ones32[:],
                        tr[(k, "t5")][:, h * NBH:(h + 1) * NBH],
                        start=True, stop=True,
                    )
                bck = sb_small.tile([P, NB], f32, tag="bc", bufs=2,
                                    name=f"bc_{k}")
                nc.vector.reciprocal_approx_fast(bck[:], sr[:, i, :])
                bc[k] = bck

            def drain_out(k):
                for lt in range(2):
                    i = slot()
                    for h in range(2):
                        nc.tensor.matmul(
                            sr[:, i, h * NBH:(h + 1) * NBH],
                            vu_bf[:, lt * P:(lt + 1) * P],
                            mscs[k][:, h * NBH:(h + 1) * NBH],
                            start=True, stop=True,
                        )
                    fin = outfin_pool.tile([P, NB], f16, tag="fin")
                    nc.vector.tensor_mul(fin[:], sr[:, i, :], bc[k][:])
                    nc.gpsimd.dma_start(
                        out_ext[lt * P:(lt + 1) * P, k * NB:(k + 1) * NB],
                        fin[:],
                    )

            # PE warm-up while the x DMA is in flight
            for _ in range(10):
                i = slot()
                nc.tensor.matmul(
                    sr[:, i, 0:NBH], wrm[:, :P], wrm[:], start=True, stop=True
                )

            # head: first QK tiles need qT/kT half-blocks 0,1 (chunk s0)
            proj_qkT_pair(qw16, qT16, 0, "ring", on_act=True)
            proj_qkT_pair(kw16, kT16, 0, "ring", on_act=False)

            # per-group PV budgets: 32 js per block, half-block lag
            BUD = [3, 3, 3, 3, 4, 3, 3, 3, 3, 4, 0]
            BUD0 = [0, 0, 0, 0, 0, 3, 3, 3, 3, 4, 0]
            BUD3 = [3, 3, 3, 3, 4, 3, 5, 6, 6, 7, 2]

            for k in range(NT):
                pvq.extend((k, j) for j in range(MT))
                bud = BUD0 if k == 0 else (BUD3 if k == NT - 1 else BUD)
                for g in range(11):
                    emit_pv(bud[g])
                    qk_exp_P(k, g)
                    if g < 10:
                        qk_exp_S(k, g)
                    if k == 0:
                        # projection fillers: w batches + late qkT halves
                        if g <= 3:
                            proj_w_batch(2 * g)
                            proj_w_batch(2 * g + 1)
                        if g == 1:
                            proj_qkT_pair(kw16, kT16, 2, "aux")
                        if g == 3:
                            proj_qkT_pair(kw16, kT16, 4, "aux")
                        if g == 5:
                            proj_qkT_pair(kw16, kT16, 6, "ring")
                        if g == 6:
                            proj_qkT_pair(qw16, qT16, 2, "ring")
                        if g == 8:
                            proj_qkT_pair(qw16, qT16, 4, "ring")
                    if k == 1 and g == 1:
                        proj_qkT_pair(qw16, qT16, 6, "ring")
                    if k >= 1:
                        if g == 0:
                            fold_last(k - 1)
                        if g == 1:
                            bc_chain(k - 1)
                        if g == 5:
                            drain_out(k - 1)
                    tree_adds(k, g)

            # epilogue: drain the PV queue, block-3 rowsum chain, output
            k3 = NT - 1
            emit_pv(len(pvq) - pvhead[0])
            fold_last(k3)
            bc_chain(k3)
            drain_out(k3)

    if not nc.is_finalized():
        nc.finalize()
    return nc


_GRAPH_CACHE = {}


def _get_graph():
    if "nc" not in _GRAPH_CACHE:
        _GRAPH_CACHE["nc"] = _build()
    return _GRAPH_CACHE["nc"]


def run(inputs: dict, trace: bool = False):
    """Run the SPMD kernel on 8 cores. Returns (output, BassKernelResults)."""
    from concourse.bass_utils import run_bass_kernel_spmd

    x = np.asarray(inputs["x"], dtype=np.float32)
    Q = np.asarray(inputs["Q"], dtype=np.float32)[0]
    K = np.asarray(inputs["K"], dtype=np.float32)[0]
    Vd = np.asarray(inputs["V_down"], dtype=np.float32)[0]
    Vu = np.asarray(inputs["V_up"], dtype=np.float32)[0]

    wq = np.ascontiguousarray(Q).astype(np.float16)
    wk = np.ascontiguousarray(K).astype(np.float16)
    vd = np.ascontiguousarray(Vd).astype(np.float16)
    vu = np.ascontiguousarray(Vu).astype(np.float16)

    in_maps = []
    for b in range(B):
        in_maps.append({
            "xT": np.ascontiguousarray(x[b].T).astype(np.float16),
            "Wq": wq,
            "Wk": wk,
            "Vd": vd,
            "Vu": vu,
        })

    nc = _get_graph()
    res = run_bass_kernel_spmd(nc, in_maps, core_ids=list(range(B)), trace=trace)
    # device output is [L, N] per core; un-transpose during the gather
    out = np.stack([np.asarray(res.results[i]["out"]).astype(np.float32).T for i in range(B)])
    return np.ascontiguousarray(out, dtype=np.float32), res


def kernel(**inputs) -> np.ndarray:
    out, _ = run(inputs, trace=False)
    return out
